# revision 1
# baseline (speedup 1.0000x reference)
"""Trainium2 Bass kernel for ClaimValidationLoss.

Data-parallel over 8 NeuronCores: each core takes 32 of the 256 batches.
Instead of streaming the full 32MB adjacency shard through the core, the
kernel computes flat element offsets for its 32K claims on-device (DVE int
ops) and gathers exactly the 32K probabilities it needs straight from DRAM
via GPSIMD indirect DMAs (one [128,1]-offset gather per claim column; the
vector-indirect SWDGE ucode honors one offset per partition per
instruction). The BCE transform runs on DVE (coefficients s,w such that
q = p*w + s covers the flip / rt>=4 / padding cases), ACT computes ln(q),
DVE reduces per-partition (sum_log_q, n_valid), and the host all-reduces
the per-partition pairs and does the final division.

Raw bacc (no TileContext): the program is a single linear dataflow, so
hand-placed semaphores avoid Tile's all-engine entry/exit barriers.
"""

import numpy as np

import concourse.bass as bass
from concourse import bacc, mybir
from concourse.bass_utils import run_bass_kernel_spmd

# Problem geometry (hardcoded per contest contract).
B, N, M = 256, 512, 1024
NCORES = 8
BL = B // NCORES            # 32 batches per core
P = 128                     # SBUF partitions
CF = BL * M // P            # 256 claims per partition
TOTAL = BL * N * N          # 8388608 adjacency elements per core
SHIFT_NN = 18               # log2(N*N)
SHIFT_BATCH = 2             # log2(P / BL): partition p holds batch p >> 2
EPS = float(np.float32(1e-7))
ONE_M_EPS = float(np.float32(1.0 - 1e-7))

f32 = mybir.dt.float32
i32 = mybir.dt.int32
Alu = mybir.AluOpType
Act = mybir.ActivationFunctionType

_CACHE = {}


def _build_nc():
    nc = bacc.Bacc("TRN2", target_bir_lowering=False, debug=False,
                   dynamic_dma_scratch_size=65536)

    adj = nc.dram_tensor("adj", [TOTAL, 1], f32, kind="ExternalInput")
    claims = nc.dram_tensor("claims", [P, 5 * CF], i32, kind="ExternalInput")
    out = nc.dram_tensor("out", [P, 2], f32, kind="ExternalOutput")

    cl = nc.alloc_sbuf_tensor("cl", [P, 5 * CF], i32)
    base = nc.alloc_sbuf_tensor("base", [P, 1], i32)
    off = nc.alloc_sbuf_tensor("off", [P, CF], i32)
    praw = nc.alloc_sbuf_tensor("praw", [P, CF], f32)
    s_t = nc.alloc_sbuf_tensor("s_t", [P, CF], f32)
    w_t = nc.alloc_sbuf_tensor("w_t", [P, CF], f32)
    q_t = nc.alloc_sbuf_tensor("q_t", [P, CF], f32)
    ai_t = nc.alloc_sbuf_tensor("ai_t", [P, CF], i32)
    is4_t = nc.alloc_sbuf_tensor("is4_t", [P, CF], i32)
    vf_t = nc.alloc_sbuf_tensor("vf_t", [P, CF], f32)
    lg_t = nc.alloc_sbuf_tensor("lg_t", [P, CF], f32)
    consts = nc.alloc_sbuf_tensor("consts", [P, 3], f32)   # [0.5, 1.0, 0.0]
    stats = nc.alloc_sbuf_tensor("stats", [P, 2], f32)     # [sum_log_q, n_valid]
    actwarm = nc.alloc_sbuf_tensor("actwarm", [P, 1], f32)

    s_ab1 = nc.alloc_semaphore("s_ab1")     # claims va|vb head chunk DMA
    s_ab = nc.alloc_semaphore("s_ab")       # claims va|vb DMA
    s_rest = nc.alloc_semaphore("s_rest")   # claims rt|tt|mk DMA
    s_base = nc.alloc_semaphore("s_base")   # base offsets ready
    s_off = nc.alloc_semaphore("s_off")     # gather offsets ready
    s_g = nc.alloc_semaphore("s_g")         # gather done (first half)
    s_g2 = nc.alloc_semaphore("s_g2")       # gather done (second half)
    s_vf = nc.alloc_semaphore("s_vf")       # n_valid column + consts ready
    s_q = nc.alloc_semaphore("s_q")         # q ready for Ln
    s_lg = nc.alloc_semaphore("s_lg")       # ln(q) tile ready
    s_ln = nc.alloc_semaphore("s_ln")       # log column ready
    s_out = nc.alloc_semaphore("s_out")     # output DMA done

    va = cl.ap()[:, 0:CF]
    vb = cl.ap()[:, CF:2 * CF]
    rt = cl.ap()[:, 2 * CF:3 * CF]
    tt = cl.ap()[:, 3 * CF:4 * CF]
    mk = cl.ap()[:, 4 * CF:5 * CF]

    # ---- SYNC/SCALAR: input DMAs on two HWDGE queues in parallel.
    # A 16-column head chunk of va/vb lands first so the serialized gather
    # stream (the kernel bottleneck) can start ~7us earlier.
    HC = 16
    nc.sync.dma_start(cl.ap()[:, 0:HC], claims.ap()[:, 0:HC]) \
        .then_inc(s_ab1, 16)
    nc.sync.dma_start(cl.ap()[:, CF:CF + HC], claims.ap()[:, CF:CF + HC]) \
        .then_inc(s_ab1, 16)
    nc.sync.dma_start(cl.ap()[:, HC:CF], claims.ap()[:, HC:CF]) \
        .then_inc(s_ab, 16)
    nc.sync.dma_start(cl.ap()[:, CF + HC:2 * CF], claims.ap()[:, CF + HC:2 * CF]) \
        .then_inc(s_ab, 16)
    nc.scalar.dma_start(cl.ap()[:, 2 * CF:5 * CF], claims.ap()[:, 2 * CF:5 * CF]) \
        .then_inc(s_rest, 16)

    # ---- SCALAR: warm the Ln activation table while DMAs run ----
    nc.scalar.activation(out=actwarm.ap()[:, :], in_=actwarm.ap()[:, :],
                         func=Act.Ln, bias=1.0, scale=0.0)   # ln(0*x+1) = 0

    # ---- GPSIMD: base[p] = p (shifted into batch*N*N on DVE below) ----
    nc.gpsimd.iota(base.ap()[:, :], pattern=[[0, 1]], base=0, channel_multiplier=1)
    nc.gpsimd.maybe_drain_then_inc((s_base, 1))

    # ---- VECTOR: constants (no deps) ----
    nc.vector.memset(consts.ap()[:, 0:1], 0.5)
    nc.vector.memset(consts.ap()[:, 1:2], 1.0)
    nc.vector.memset(consts.ap()[:, 2:3], 0.0)

    # ---- VECTOR: offsets once va|vb there ----
    nc.vector.wait_ge(s_base, 1)
    nc.vector.tensor_scalar(out=base.ap()[:, :], in0=base.ap()[:, :],
                            scalar1=SHIFT_BATCH, scalar2=SHIFT_NN,
                            op0=Alu.arith_shift_right, op1=Alu.logical_shift_left)
    nc.vector.wait_ge(s_ab1, 32)
    nc.vector.scalar_tensor_tensor(out=off.ap()[:, 0:HC], in0=cl.ap()[:, 0:HC],
                                   scalar=N, in1=cl.ap()[:, CF:CF + HC],
                                   op0=Alu.mult, op1=Alu.add)
    nc.vector.drain()
    nc.vector.tensor_tensor(out=off.ap()[:, 0:HC], in0=off.ap()[:, 0:HC],
                            in1=base.ap()[:, 0:1].to_broadcast([P, HC]),
                            op=Alu.add)
    nc.vector.maybe_drain_then_inc((s_off, 1))
    nc.vector.wait_ge(s_ab, 32)
    nc.vector.scalar_tensor_tensor(out=off.ap()[:, HC:CF],
                                   in0=cl.ap()[:, HC:CF], scalar=N,
                                   in1=cl.ap()[:, CF + HC:2 * CF],
                                   op0=Alu.mult, op1=Alu.add)
    nc.vector.drain()
    nc.vector.tensor_tensor(out=off.ap()[:, HC:CF], in0=off.ap()[:, HC:CF],
                            in1=base.ap()[:, 0:1].to_broadcast([P, CF - HC]),
                            op=Alu.add)
    nc.vector.maybe_drain_then_inc((s_off, 1))

    # ---- GPSIMD: the gather. The vector-indirect SWDGE ucode consumes ONE
    # offset per partition per instruction (multi-index offset APs silently
    # misbehave on HW), so issue one [128,1] gather per claim column.
    nc.gpsimd.wait_ge(s_off, 1)
    for k in range(CF):
        if k == HC:
            nc.gpsimd.wait_ge(s_off, 2)
        nc.gpsimd.indirect_dma_start(
            out=praw.ap()[:, k:k + 1], out_offset=None, in_=adj.ap()[:, :],
            in_offset=bass.IndirectOffsetOnAxis(ap=off.ap()[:, k:k + 1], axis=0),
            oob_is_err=False) \
            .then_inc(s_g if k < CF // 2 else s_g2, 16)

    # ---- VECTOR: coefficient prep under the gather.
    # q = praw*w + s reproduces every case:
    #   normal claims:  s = (rt&1 == is_true), w = 1-2s  -> q = p or 1-p
    #   rt >= 4:        s = 0.5, w = 0                   -> q = 0.5
    #   padded:         s = 1,   w = 0                   -> q = 1, ln q ~ 0
    nc.vector.wait_ge(s_rest, 16)
    nc.vector.tensor_scalar(out=ai_t.ap()[:, :], in0=rt, scalar1=1, scalar2=None,
                            op0=Alu.bitwise_and)
    nc.vector.tensor_scalar(out=is4_t.ap()[:, :], in0=rt, scalar1=4, scalar2=None,
                            op0=Alu.is_ge)
    nc.vector.tensor_scalar(out=vf_t.ap()[:, :], in0=mk, scalar1=0, scalar2=None,
                            op0=Alu.is_equal)
    nc.vector.drain()
    nc.vector.tensor_tensor(out=s_t.ap()[:, :], in0=ai_t.ap()[:, :], in1=tt,
                            op=Alu.is_equal)
    nc.vector.tensor_reduce(out=stats.ap()[:, 1:2], in_=vf_t.ap()[:, :],
                            axis=mybir.AxisListType.X, op=Alu.add)
    nc.vector.drain()
    nc.vector.copy_predicated(out=s_t.ap()[:, :], mask=is4_t.ap()[:, :],
                              data=consts.ap()[:, 0:1].to_broadcast([P, CF]))
    nc.vector.drain()
    nc.vector.tensor_scalar(out=w_t.ap()[:, :], in0=s_t.ap()[:, :],
                            scalar1=-2.0, scalar2=1.0,
                            op0=Alu.mult, op1=Alu.add)
    nc.vector.drain()
    nc.vector.copy_predicated(out=w_t.ap()[:, :], mask=mk,
                              data=consts.ap()[:, 2:3].to_broadcast([P, CF]))
    nc.vector.copy_predicated(out=s_t.ap()[:, :], mask=mk,
                              data=consts.ap()[:, 1:2].to_broadcast([P, CF]))
    nc.vector.maybe_drain_then_inc((s_vf, 1))

    # ---- VECTOR/SCALAR: q and ln(q) in halves; the first half's math runs
    # while the second half of the gather stream is still issuing ----
    H = CF // 2
    for h, (lo, hi) in enumerate([(0, H), (H, CF)]):
        nc.vector.wait_ge(s_g if h == 0 else s_g2, 16 * H)
        nc.vector.tensor_tensor(out=q_t.ap()[:, lo:hi], in0=praw.ap()[:, lo:hi],
                                in1=w_t.ap()[:, lo:hi], op=Alu.mult)
        nc.vector.drain()
        nc.vector.tensor_tensor(out=q_t.ap()[:, lo:hi], in0=q_t.ap()[:, lo:hi],
                                in1=s_t.ap()[:, lo:hi], op=Alu.add)
        nc.vector.drain()
        nc.vector.tensor_scalar(out=q_t.ap()[:, lo:hi], in0=q_t.ap()[:, lo:hi],
                                scalar1=EPS, scalar2=ONE_M_EPS,
                                op0=Alu.max, op1=Alu.min)
        nc.vector.maybe_drain_then_inc((s_q, 1))

        nc.scalar.wait_ge(s_q, h + 1)
        nc.scalar.activation(out=lg_t.ap()[:, lo:hi], in_=q_t.ap()[:, lo:hi],
                             func=Act.Ln)
        nc.scalar.maybe_drain_then_inc((s_lg, 1))

    # ---- VECTOR: stats[:,0] = sum_k lg (full-fp32 DVE reduce; the ACT
    # accumulator loses precision) ----
    nc.vector.wait_ge(s_lg, 2)
    nc.vector.tensor_reduce(out=stats.ap()[:, 0:1], in_=lg_t.ap()[:, :],
                            axis=mybir.AxisListType.X, op=Alu.add)
    nc.vector.maybe_drain_then_inc((s_ln, 1))

    # ---- SYNC: ship per-partition stats; host does the tiny all-reduce ----
    nc.sync.wait_ge(s_ln, 1)
    nc.sync.wait_ge(s_vf, 1)
    nc.sync.dma_start(out.ap()[:, :], stats.ap()[:, :]).then_inc(s_out, 16)
    nc.sync.wait_ge(s_out, 16)

    nc.compile()
    return nc


def kernel(posterior_adjacency, var_a, var_b, relation_type, is_true, claim_mask):
    adj = np.asarray(posterior_adjacency, dtype=np.float32)
    va = np.asarray(var_a, dtype=np.int32)
    vb = np.asarray(var_b, dtype=np.int32)
    rt = np.asarray(relation_type, dtype=np.int32)
    tt = np.asarray(is_true, dtype=np.int32)
    mk = np.asarray(claim_mask).astype(np.int32)

    if "nc" not in _CACHE:
        _CACHE["nc"] = _build_nc()
    nc = _CACHE["nc"]

    in_maps = []
    for c in range(NCORES):
        sl = slice(c * BL, (c + 1) * BL)
        in_maps.append({
            "adj": np.ascontiguousarray(adj[sl]).reshape(TOTAL, 1),
            "claims": np.concatenate(
                [va[sl].reshape(P, CF), vb[sl].reshape(P, CF),
                 rt[sl].reshape(P, CF), tt[sl].reshape(P, CF),
                 mk[sl].reshape(P, CF)], axis=1),
        })

    res = run_bass_kernel_spmd(nc, in_maps, core_ids=list(range(NCORES)))
    pairs = np.stack([r["out"] for r in res.results]).astype(np.float64)
    sum_log_q = pairs[:, :, 0].sum()
    n_valid = pairs[:, :, 1].sum()
    if n_valid > 0:
        loss = -sum_log_q / max(n_valid, 1.0)
    else:
        loss = 0.0
    return np.float32(loss)



# revision 2
# speedup vs baseline: 1.2378x; 1.2378x over previous
"""Trainium2 Bass kernel for ClaimValidationLoss.

Data-parallel over 8 NeuronCores: each core takes 32 of the 256 batches
(32768 claims, 32MB adjacency shard).

The baseline gathered one probability per claim via GPSIMD indirect DMAs
([128,1] out = 128 descriptors/instruction), paying the ~1us SWDGE fixed
overhead 256 times (~270us serialized on the Pool engine). This kernel
instead uses the SWDGE dma_gather ucode (InstDMAGatherAnt), which packs
up to num_idxs descriptors into ONE Pool instruction (994ns + 0.34ns/desc)
at 256-byte granularity:

  * Per-core flat claim offset o = b_local*2^18 + va*512 + vb. dma_gather
    indices are int16 (<32768) over 64-float blocks, so one gather can
    address 2^21 elements = exactly 8 batches. The window split is STATIC:
    window w = local batches [8w, 8w+8), 8192 claims each.
  * idx = va*8 + (vb2 >> 6) where vb2 = vb + 2^18*(batch-within-window)
    (static offset folded on the host); max = 511*8 + 4095 + 7*4096 = 32767.
  * 4 dma_gather instructions (single_packet=False; the 64KB single-packet
    SDMA limit hangs at >4096 descriptors) -> G_w[128, 64, 64] f32, claim
    slot i at partition i%128, group i//128.
  * The claim's element sits at position c = vb & 63 inside its block.
    Extraction = 6 in-place copy_predicated halving steps on DVE (binary
    select on the bits of c), ~4us/window, leaving the value at [:, :, 0].
  * BCE coefficients (s, w with q = p*w + s covering flip / rt>=4 / padding)
    prep on DVE under the gather shadow, Ln on ACT, fp32 reduces, and the
    host all-reduces the per-partition (sum_log_q, n_valid) pairs.
"""

import numpy as np

import concourse.bass as bass  # noqa: F401  (IndirectOffsetOnAxis unused now)
from concourse import bacc, mybir
from concourse.bass_utils import run_bass_kernel_spmd

# Problem geometry (hardcoded per contest contract).
B, N, M = 256, 512, 1024
NCORES = 8
BL = B // NCORES            # 32 batches per core
P = 128                     # SBUF partitions
TC = BL * M                 # 32768 claims per core
NW = 4                      # gather windows (8 batches each; int16 idx limit)
CW = TC // NW               # 8192 claims per window
GW = CW // P                # 64 claim groups per window
CF = NW * GW                # 256 claim columns in [128, 256] layout
NBLK = 32768                # 64-elem blocks per window
EPS = float(np.float32(1e-7))
ONE_M_EPS = float(np.float32(1.0 - 1e-7))

f32 = mybir.dt.float32
i32 = mybir.dt.int32
i16 = mybir.dt.int16
Alu = mybir.AluOpType
Act = mybir.ActivationFunctionType

_CACHE = {}


def _build_nc():
    nc = bacc.Bacc("TRN2", target_bir_lowering=False, debug=False,
                   dynamic_dma_scratch_size=65536)

    # Per-window adjacency shards: [32768 blocks, 64 floats].
    adjw = [nc.dram_tensor(f"adj{w}", [NBLK, 64], f32, kind="ExternalInput")
            for w in range(NW)]
    # Claim meta in window-slot layout [128, 4*CF]: vb | rt | tt | mk,
    # claim i of window w at (p = i%128, col = GW*w + i//128).
    claims = nc.dram_tensor("claims", [P, 4 * CF], i32, kind="ExternalInput")
    # va and vb2 in the SWDGE wrapped layout, host-replicated to all 8
    # GPSIMD core groups: claim i of window w at (16c + i%16, 128*w + i//16).
    wrap = nc.dram_tensor("wrap", [P, 2 * (TC // 16)], i32,
                          kind="ExternalInput")
    out = nc.dram_tensor("out", [P, 2], f32, kind="ExternalOutput")

    cl = nc.alloc_sbuf_tensor("cl", [P, 4 * CF], i32)
    wr = nc.alloc_sbuf_tensor("wr", [P, 2 * (TC // 16)], i32)
    t1 = nc.alloc_sbuf_tensor("t1", [P, TC // 16], i32)
    idxr = nc.alloc_sbuf_tensor("idxr", [P, TC // 16], i16)
    G = nc.alloc_sbuf_tensor("G", [P, NW * GW * 64], f32)
    praw = nc.alloc_sbuf_tensor("praw", [P, CF], f32)
    mb_t = nc.alloc_sbuf_tensor("mb_t", [P, 6 * CF], i32)
    s_tt = nc.alloc_sbuf_tensor("s_tt", [P, CF], f32)
    w_tt = nc.alloc_sbuf_tensor("w_tt", [P, CF], f32)
    q_t = nc.alloc_sbuf_tensor("q_t", [P, CF], f32)
    lg_t = nc.alloc_sbuf_tensor("lg_t", [P, CF], f32)
    ai_t = nc.alloc_sbuf_tensor("ai_t", [P, CF], i32)
    is4_t = nc.alloc_sbuf_tensor("is4_t", [P, CF], i32)
    vf_t = nc.alloc_sbuf_tensor("vf_t", [P, CF], f32)
    consts = nc.alloc_sbuf_tensor("consts", [P, 3], f32)  # [0.5, 1.0, 0.0]
    stats = nc.alloc_sbuf_tensor("stats", [P, 2], f32)    # [sum_log_q, n_valid]
    actwarm = nc.alloc_sbuf_tensor("actwarm", [P, 1], f32)

    s_wr = nc.alloc_semaphore("s_wr")       # wrap DMA done
    s_cl = nc.alloc_semaphore("s_cl")       # claims DMA done
    s_idx = nc.alloc_semaphore("s_idx")     # idx16 ready
    s_g = [nc.alloc_semaphore(f"s_g{w}") for w in range(NW)]
    s_vf = nc.alloc_semaphore("s_vf")       # BCE coeffs + n_valid ready
    s_q = nc.alloc_semaphore("s_q")         # per-window q ready
    s_lg = nc.alloc_semaphore("s_lg")       # per-window ln done
    s_ln = nc.alloc_semaphore("s_ln")       # final log reduce done
    s_out = nc.alloc_semaphore("s_out")     # output DMA done

    vb = cl.ap()[:, 0:CF]
    rt = cl.ap()[:, CF:2 * CF]
    tt = cl.ap()[:, 2 * CF:3 * CF]
    mk = cl.ap()[:, 3 * CF:4 * CF]
    vaw = wr.ap()[:, 0:TC // 16]
    vbw = wr.ap()[:, TC // 16:2 * (TC // 16)]

    # ---- input DMAs: wrap (gates the gather pipeline) on sync, claims on
    # scalar so they stream in parallel on separate HWDGE queues.
    nc.sync.dma_start(wr.ap()[:, :], wrap.ap()[:, :]).then_inc(s_wr, 16)
    nc.scalar.dma_start(cl.ap()[:, :], claims.ap()[:, :]).then_inc(s_cl, 16)

    # ---- SCALAR: warm the Ln activation table while DMAs run ----
    nc.scalar.activation(out=actwarm.ap()[:, :], in_=actwarm.ap()[:, :],
                         func=Act.Ln, bias=1.0, scale=0.0)   # ln(0*x+1) = 0

    # ---- VECTOR: constants (no deps) ----
    nc.vector.memset(consts.ap()[:, 0:1], 0.5)
    nc.vector.memset(consts.ap()[:, 1:2], 1.0)
    nc.vector.memset(consts.ap()[:, 2:3], 0.0)

    # ---- VECTOR: gather indices. idx = va*8 + (vb2 >> 6), already in the
    # wrapped+replicated layout, written as int16.
    nc.vector.wait_ge(s_wr, 16)
    nc.vector.tensor_scalar(out=t1.ap()[:, :], in0=vbw,
                            scalar1=6, scalar2=None,
                            op0=Alu.arith_shift_right)
    nc.vector.drain()
    nc.vector.scalar_tensor_tensor(out=t1.ap()[:, :], in0=vaw, scalar=8,
                                   in1=t1.ap()[:, :],
                                   op0=Alu.mult, op1=Alu.add)
    nc.vector.drain()
    nc.vector.tensor_scalar(out=idxr.ap()[:, :], in0=t1.ap()[:, :],
                            scalar1=0, scalar2=None, op0=Alu.add)
    nc.vector.maybe_drain_then_inc((s_idx, 1))

    # ---- GPSIMD: the four window gathers. Each is ONE SWDGE instruction
    # with 8192 descriptors of 256B. Window w+2 gated on window w's DMA
    # completion to bound in-flight descriptors.
    nc.gpsimd.wait_ge(s_idx, 1)
    for w in range(NW):
        if w >= 2:
            nc.gpsimd.wait_ge(s_g[w - 2], 16)
        nc.gpsimd.dma_gather(
            out_ap=G.ap()[:, w * GW * 64:(w + 1) * GW * 64]
                    .rearrange("p (g e) -> p g e", e=64),
            in_ap=adjw[w].ap()[:, :],
            idxs_ap=idxr.ap()[:, w * (CW // 16):(w + 1) * (CW // 16)],
            num_idxs=CW,
            num_idxs_reg=CW,
            elem_size=64,
            transpose=False,
            single_packet=False,
        ).then_inc(s_g[w], 16)

    # ---- VECTOR: bit masks of c = vb & 63 (for the extraction selects)
    # and BCE coefficient prep, all under the gather shadow.
    nc.vector.wait_ge(s_cl, 16)
    for b in range(6):
        nc.vector.tensor_scalar(out=mb_t.ap()[:, b * CF:(b + 1) * CF],
                                in0=vb, scalar1=1 << b, scalar2=None,
                                op0=Alu.bitwise_and)
    nc.vector.tensor_scalar(out=ai_t.ap()[:, :], in0=rt, scalar1=1,
                            scalar2=None, op0=Alu.bitwise_and)
    nc.vector.tensor_scalar(out=is4_t.ap()[:, :], in0=rt, scalar1=4,
                            scalar2=None, op0=Alu.is_ge)
    nc.vector.tensor_scalar(out=vf_t.ap()[:, :], in0=mk, scalar1=0,
                            scalar2=None, op0=Alu.is_equal)
    nc.vector.drain()
    nc.vector.tensor_tensor(out=s_tt.ap()[:, :], in0=ai_t.ap()[:, :], in1=tt,
                            op=Alu.is_equal)
    nc.vector.tensor_reduce(out=stats.ap()[:, 1:2], in_=vf_t.ap()[:, :],
                            axis=mybir.AxisListType.X, op=Alu.add)
    nc.vector.drain()
    nc.vector.copy_predicated(out=s_tt.ap()[:, :], mask=is4_t.ap()[:, :],
                              data=consts.ap()[:, 0:1].to_broadcast([P, CF]))
    nc.vector.drain()
    nc.vector.tensor_scalar(out=w_tt.ap()[:, :], in0=s_tt.ap()[:, :],
                            scalar1=-2.0, scalar2=1.0,
                            op0=Alu.mult, op1=Alu.add)
    nc.vector.drain()
    nc.vector.copy_predicated(out=w_tt.ap()[:, :], mask=mk,
                              data=consts.ap()[:, 2:3].to_broadcast([P, CF]))
    nc.vector.copy_predicated(out=s_tt.ap()[:, :], mask=mk,
                              data=consts.ap()[:, 1:2].to_broadcast([P, CF]))
    nc.vector.maybe_drain_then_inc((s_vf, 1))

    # ---- VECTOR/SCALAR per window: extract claim element via 6 halving
    # selects on the bits of c, then q = p*w + s, clamp; ACT does ln(q).
    for w in range(NW):
        G3 = G.ap()[:, w * GW * 64:(w + 1) * GW * 64] \
              .rearrange("p (g e) -> p g e", e=64)
        cols = slice(w * GW, (w + 1) * GW)
        nc.vector.wait_ge(s_g[w], 16)
        for b in range(5, -1, -1):
            hw = 1 << b
            mask = mb_t.ap()[:, b * CF + w * GW:b * CF + (w + 1) * GW] \
                       .unsqueeze(2).to_broadcast([P, GW, hw])
            nc.vector.copy_predicated(out=G3[:, :, 0:hw], mask=mask,
                                      data=G3[:, :, hw:2 * hw])
            nc.vector.drain()
        g0 = G3[:, :, 0:1].squeeze(2)
        nc.vector.wait_ge(s_vf, 1)
        nc.vector.tensor_tensor(out=q_t.ap()[:, cols], in0=g0,
                                in1=w_tt.ap()[:, cols], op=Alu.mult)
        nc.vector.drain()
        nc.vector.tensor_tensor(out=q_t.ap()[:, cols],
                                in0=q_t.ap()[:, cols],
                                in1=s_tt.ap()[:, cols], op=Alu.add)
        nc.vector.drain()
        nc.vector.tensor_scalar(out=q_t.ap()[:, cols],
                                in0=q_t.ap()[:, cols],
                                scalar1=EPS, scalar2=ONE_M_EPS,
                                op0=Alu.max, op1=Alu.min)
        nc.vector.maybe_drain_then_inc((s_q, 1))

        nc.scalar.wait_ge(s_q, w + 1)
        nc.scalar.activation(out=lg_t.ap()[:, cols], in_=q_t.ap()[:, cols],
                             func=Act.Ln)
        nc.scalar.maybe_drain_then_inc((s_lg, 1))

    # ---- VECTOR: stats[:,0] = sum of ln(q) (full-fp32 DVE reduce) ----
    nc.vector.wait_ge(s_lg, NW)
    nc.vector.tensor_reduce(out=stats.ap()[:, 0:1], in_=lg_t.ap()[:, :],
                            axis=mybir.AxisListType.X, op=Alu.add)
    nc.vector.maybe_drain_then_inc((s_ln, 1))

    # ---- SYNC: ship per-partition stats; host does the tiny all-reduce ----
    nc.sync.wait_ge(s_ln, 1)
    nc.sync.wait_ge(s_vf, 1)
    nc.sync.dma_start(out.ap()[:, :], stats.ap()[:, :]).then_inc(s_out, 16)
    nc.sync.wait_ge(s_out, 16)

    nc.compile()
    return nc


def _prep_core_inputs(adj, va, vb, rt, tt, mk, c):
    """Build one core's input map (batches [32c, 32c+32))."""
    sl = slice(c * BL, (c + 1) * BL)
    adj_c = adj[sl].reshape(NW, NBLK, 64)

    def claim_layout(F):
        # claim i of window w -> (p = i%128, col = GW*w + i//128)
        X = F[sl].reshape(NW, GW, P)
        return np.concatenate([X[w].T for w in range(NW)], axis=1)

    def wrap_layout(F):
        # claim i of window w -> (i%16, 128*w + i//16), replicated x8
        X = F.reshape(NW, CW // 16, 16)
        W16 = np.concatenate([X[w].T for w in range(NW)], axis=1)
        return np.tile(W16, (8, 1))

    va_c = va[sl].reshape(TC)
    # vb2 folds the static window-local batch offset: claim i belongs to
    # local batch i//1024, i.e. batch (i//1024)%8 of its window.
    vb2_c = vb[sl].reshape(TC) + (1 << 18) * ((np.arange(TC) // M) % (BL // NW))

    in_map = {f"adj{w}": adj_c[w] for w in range(NW)}
    in_map["claims"] = np.concatenate(
        [claim_layout(vb), claim_layout(rt), claim_layout(tt),
         claim_layout(mk)], axis=1)
    in_map["wrap"] = np.concatenate(
        [wrap_layout(va_c), wrap_layout(vb2_c)], axis=1)
    return in_map


def kernel(posterior_adjacency, var_a, var_b, relation_type, is_true,
           claim_mask):
    adj = np.asarray(posterior_adjacency, dtype=np.float32)
    va = np.asarray(var_a, dtype=np.int32)
    vb = np.asarray(var_b, dtype=np.int32)
    rt = np.asarray(relation_type, dtype=np.int32)
    tt = np.asarray(is_true, dtype=np.int32)
    mk = np.asarray(claim_mask).astype(np.int32)

    if "nc" not in _CACHE:
        _CACHE["nc"] = _build_nc()
    nc = _CACHE["nc"]

    in_maps = [_prep_core_inputs(adj, va, vb, rt, tt, mk, c)
               for c in range(NCORES)]

    res = run_bass_kernel_spmd(nc, in_maps, core_ids=list(range(NCORES)))
    pairs = np.stack([r["out"] for r in res.results]).astype(np.float64)
    sum_log_q = pairs[:, :, 0].sum()
    n_valid = pairs[:, :, 1].sum()
    if n_valid > 0:
        loss = -sum_log_q / max(n_valid, 1.0)
    else:
        loss = 0.0
    return np.float32(loss)
